# revision 1
# baseline (speedup 1.0000x reference)
import sys, os
sys.path.insert(0, '/opt/trn_rl_repo')
import numpy as np
import ml_dtypes

import concourse.bass as bass
import concourse.mybir as mybir
import concourse.tile as tile
from concourse import bacc
from concourse.bass_utils import run_bass_kernel_spmd

F32 = mybir.dt.float32
BF16 = mybir.dt.bfloat16
FP8 = mybir.dt.float8e4
PM = mybir.MatmulPerfMode.DoubleRow
AF = mybir.ActivationFunctionType
OP = mybir.AluOpType
F8 = ml_dtypes.float8_e4m3fn
BF = ml_dtypes.bfloat16

B, DIM, HEADS, SR, RES, HID = 16, 256, 8, 7, 56, 1024
N = RES * RES              # 3136
NP = 3200                  # padded token count (25*128) for DMA transpose
LN_EPS, BN_EPS = 1e-6, 1e-5
NCORES = 8
BPC = B // NCORES
NT = 25
SW = 16.0                  # fp8 weight rescale

_CACHE = {}


def _build():
    nc = bacc.Bacc(None, target_bir_lowering=False, debug=True)

    xg = nc.dram_tensor([BPC, N, DIM], BF16, kind="ExternalInput")
    out = nc.dram_tensor([BPC, N, DIM], F32, kind="ExternalOutput")
    G_d = nc.dram_tensor([64, N], FP8, kind="ExternalInput")
    Wv_d = nc.dram_tensor([128, 2, 2, 128], FP8, kind="ExternalInput")
    Wp_d = nc.dram_tensor([128, 2, 2, 128], FP8, kind="ExternalInput")
    F1_d = nc.dram_tensor([128, 2, 8, 128], FP8, kind="ExternalInput")
    F3_d = nc.dram_tensor([128, 4, 2, 2, 128], FP8, kind="ExternalInput")
    dgp_d = nc.dram_tensor([128, 14, 3, 2, 128], FP8, kind="ExternalInput")
    dgs_d = nc.dram_tensor([128, 14, 3, 128], FP8, kind="ExternalInput")
    # per-partition columns: ln g/b not needed (folded); drain scales/biases
    cols_d = nc.dram_tensor([128, 24], F32, kind="ExternalInput")
    colff_d = nc.dram_tensor([128, 4, 8], F32, kind="ExternalInput")

    # cols layout (k index):
    # 0,1: B_v ; 2,3: inv1/S ; 4,5: beta1 ; 6,7: invq1 = inv1/(49 S) ;
    # 8,9: inv2/S ; 10,11: beta2 ; 12,13: invq2 ; 14,15: invv/S ;
    # 16,17: B_p ; 18,19: invf3/S ; 20,21: betaf3 ; 22: unused, 23: unused
    # colf1: [:, 0, :]=invf1/S  [:, 1, :]=betaf1'

    with tile.TileContext(nc) as tc:
        with (
            tc.tile_pool(name="cst", bufs=1) as cst,
            tc.tile_pool(name="big", bufs=1) as big,
            tc.tile_pool(name="sm", bufs=1) as sm,
            tc.tile_pool(name="tmp", bufs=3) as tmp,
            tc.tile_pool(name="pps", bufs=8, space="PSUM") as pps,
        ):
            G = cst.tile([64, N], FP8)
            nc.sync.dma_start(out=G, in_=G_d[:])
            Wv = cst.tile([128, 2, 2, 128], FP8)
            nc.sync.dma_start(out=Wv, in_=Wv_d[:])
            Wp = cst.tile([128, 2, 2, 128], FP8)
            nc.sync.dma_start(out=Wp, in_=Wp_d[:])
            F1 = cst.tile([128, 2, 8, 128], FP8)
            nc.sync.dma_start(out=F1, in_=F1_d[:])
            F3 = cst.tile([128, 4, 2, 2, 128], FP8)
            nc.sync.dma_start(out=F3, in_=F3_d[:])
            dgp = cst.tile([128, 14, 3, 2, 128], FP8)
            nc.sync.dma_start(out=dgp, in_=dgp_d[:])
            dgs = cst.tile([128, 14, 3, 128], FP8)
            nc.sync.dma_start(out=dgs, in_=dgs_d[:])
            cols = cst.tile([128, 24], F32)
            nc.sync.dma_start(out=cols, in_=cols_d[:])
            colff = cst.tile([128, 4, 8], F32)
            nc.sync.dma_start(out=colff, in_=colff_d[:])
            epsln = cst.tile([128, 1], F32)
            nc.vector.memset(epsln, LN_EPS)

            # persistent padded conv-input buffers (borders stay zero)
            v_pad = cst.tile([128, 2, 58, 64], FP8)
            nc.vector.memset(v_pad.bitcast(mybir.dt.int32), 0)
            s1_pad = cst.tile([128, 2, 58, 64], FP8)
            nc.vector.memset(s1_pad.bitcast(mybir.dt.int32), 0)
            yu_pad = cst.tile([128, 2, 58, 64], FP8)
            nc.vector.memset(yu_pad.bitcast(mybir.dt.int32), 0)
            z1_pad = cst.tile([128, 4, 58, 64], FP8)
            nc.vector.memset(z1_pad.bitcast(mybir.dt.int32), 0)
            v_aug = cst.tile([128, NT, 8, 33], BF16)
            nc.vector.memset(v_aug[:, :, :, 32:33], 1.0)

            def col(k):
                return cols[:, k:k + 1]

            def ln_to_ct(x_tok, xn_tok, xn_cti, xn_fp8):
                """token-major LN stats+normalize -> bf16 xn_tok; DMA-T -> interleaved
                bf16 xn_cti; scalar cast -> plain fp8 [128,2,NP]."""
                for t in range(NT):
                    rows = 128 if t < NT - 1 else 64
                    st = tmp.tile([128, 6], F32, tag="st", bufs=4, name="st")
                    nc.vector.bn_stats(out=st[:rows], in_=x_tok[:rows, t, :])
                    mv = tmp.tile([128, 2], F32, tag="mv", bufs=4, name="mv")
                    nc.vector.bn_aggr(out=mv[:rows], in_=st[:rows])
                    sd = tmp.tile([128, 1], F32, tag="sd", bufs=4, name="sd")
                    nc.scalar.activation(out=sd[:rows], in_=mv[:rows, 1:2],
                                         func=AF.Sqrt, bias=epsln[:rows])
                    rs = tmp.tile([128, 1], F32, tag="rs", bufs=4, name="rs")
                    nc.vector.reciprocal(out=rs[:rows], in_=sd[:rows])
                    nc.vector.tensor_scalar(out=xn_tok[:rows, t, :], in0=x_tok[:rows, t, :],
                                            scalar1=mv[:rows, 0:1], scalar2=rs[:rows],
                                            op0=OP.subtract, op1=OP.mult)
                srcv = xn_cti.rearrange("p (t k j) -> p t k j", k=2, j=128)
                for q0, q1 in ((0, 7), (7, 13), (13, 19), (19, 25)):
                    nc.sync.dma_start_transpose(
                        out=xn_cti[:, q0 * 256:q1 * 256].rearrange(
                            "p (m j) -> p m j", j=128),
                        in_=xn_tok[:, q0:q1, :].rearrange("p t c -> p (t c)"))
                    tend = min(q1, 24)
                    nc.scalar.copy(
                        out=xn_fp8[:, :, q0 * 128:tend * 128].rearrange(
                            "p k (t j) -> p k t j", j=128),
                        in_=srcv[:, q0:tend].rearrange("p t k j -> p k t j"))
                nc.scalar.copy(out=xn_fp8[:, :, 3072:3136], in_=srcv[:, 24, :, 0:64])

            def conv(pad_ch, ci, nrows, drain):
                """fp8-DR conv on padded [58,64] image chunk. pairs ((0,x),(1,x)) + singles
                (2,x). nrows: stripe rows (7 for pooled convs, 8 otherwise)."""
                nstripe = RES // nrows
                for s in range(nstripe):
                    r0 = s * nrows
                    cp = pps.tile([128, nrows, RES], F32, tag="cp", bufs=3,
                                  name="cp")
                    for x in range(3):
                        base = pad_ch[:, r0:r0 + nrows, x:x + RES]
                        rhs = base.copy()
                        rhs.ap.insert(1, (64, 2))
                        nc.tensor.matmul(cp, dgp[:, ci, x, :, :], rhs,
                                         start=(x == 0), stop=False, perf_mode=PM)
                    for x in range(3):
                        base = pad_ch[:, r0 + 2:r0 + 2 + nrows, x:x + RES]
                        nc.tensor.matmul(cp, dgs[:, ci, x, :], base,
                                         start=False, stop=(x == 2))
                    drain(s, r0, cp)

            def pool_from(cp, pool_out, ch, s, sc_k):
                t1 = tmp.tile([128, 7, 8], F32, tag="t1", bufs=4, name="t1")
                nc.vector.tensor_reduce(
                    out=t1, in_=cp.rearrange("p h (wb k) -> p h wb k", k=7),
                    axis=mybir.AxisListType.X, op=OP.add)
                t2 = tmp.tile([128, 8], F32, tag="t2", bufs=4, name="t2")
                nc.vector.tensor_reduce(
                    out=t2, in_=t1.rearrange("p h w -> p w h"),
                    axis=mybir.AxisListType.X, op=OP.add)
                nc.vector.tensor_scalar(out=pool_out[:, ch, s, :], in0=t2,
                                        scalar1=col(sc_k + ch), scalar2=col(sc_k - 2 + ch),
                                        op0=OP.mult, op1=OP.add)

            S = [dict() for _ in range(BPC)]

            def st_load_ln1(b):
                d = S[b]
                x_tok = big.tile([128, NT, 256], BF16, tag="x_tok", bufs=2,
                                 name=f"x_tok{b}")
                nc.sync.dma_start(
                    out=x_tok[:, 0:24, :],
                    in_=xg[b, 0:3072, :].rearrange("(t p) c -> p t c", p=128))
                nc.sync.dma_start(out=x_tok[:64, 24, :], in_=xg[b, 3072:3136, :])
                xn_tok = big.tile([128, NT, 256], BF16, tag="xn_tok", bufs=2,
                                  name=f"xn_tok{b}")
                xn_cti = big.tile([128, NT * 256], BF16, tag="shA", name=f"xn_cti{b}")
                xn_fp8 = big.tile([128, 2, N], FP8, tag="shC", name=f"xn_fp8{b}")
                ln_to_ct(x_tok, xn_tok, xn_cti, xn_fp8)
                d.update(x_tok=x_tok, xn_fp8=xn_fp8)

            def st_v_convs(b):
                d = S[b]
                v_dense = big.tile([128, 2, NP], BF16, tag="shA", name=f"v_dense{b}")
                for mc in range(2):
                    for s in range(7):
                        pv = pps.tile([128, 448], F32, tag="pgA", bufs=2, name="pv")
                        nc.tensor.matmul(pv, Wv[:, :, mc, :],
                                         d["xn_fp8"][:, :, s * 448:(s + 1) * 448],
                                         start=True, stop=True, perf_mode=PM)
                        nc.vector.tensor_scalar(
                            out=v_dense[:, mc, s * 448:(s + 1) * 448], in0=pv,
                            scalar1=1.0 / SW, scalar2=col(0 + mc),
                            op0=OP.mult, op1=OP.add)
                        nc.vector.tensor_scalar(
                            out=v_pad[:, mc, 1 + 8 * s:9 + 8 * s, 1:57], in0=pv,
                            scalar1=1.0 / SW, scalar2=col(0 + mc),
                            op0=OP.mult, op1=OP.add)
                qv = sm.tile([128, 2, 8, 8], F32, tag="qv", name=f"qv{b}")
                for ch in range(2):
                    def drain1(s, r0, cp, ch=ch):
                        pool_from(cp, qv, ch, s, 6)
                        nc.scalar.activation(
                            out=s1_pad[:, ch, 1 + r0:1 + r0 + 7, 1:57],
                            in_=cp, func=AF.Gelu, scale=col(2 + ch), bias=col(4 + ch))
                    conv(v_pad[:, ch], 0 + ch, 7, drain1)
                kv = sm.tile([128, 2, 8, 8], F32, tag="kv", name=f"kv{b}")
                skip2 = big.tile([128, 2, NP], BF16, tag="xn_tok", bufs=2,
                                 name=f"skip2{b}")
                for ch in range(2):
                    def drain2(s, r0, cp, ch=ch):
                        pool_from(cp, kv, ch, s, 12)
                        nc.scalar.activation(
                            out=skip2[:, ch, r0 * RES:(r0 + 7) * RES],
                            in_=cp, func=AF.Gelu, scale=col(8 + ch), bias=col(10 + ch))
                    conv(s1_pad[:, ch], 2 + ch, 7, drain2)
                d.update(v_dense=v_dense, qv=qv, kv=kv, skip2=skip2)

            def st_attn(b):
                d = S[b]
                v_tok = big.tile([128, 2, NT, 128], BF16, tag="v_tok", name=f"v_tok{b}")
                for kc in range(2):
                    nc.sync.dma_start_transpose(out=v_tok[:, kc], in_=d["v_dense"][:, kc])
                for t in range(NT):
                    nc.vector.tensor_copy(
                        out=v_aug[:, t, :, 0:32].rearrange("p (k f) d -> p k f d", k=2),
                        in_=v_tok[:, :, t, :].rearrange("p k (f d) -> p k f d", d=32))
                qb = sm.tile([128, 2, 64], BF16, tag="qb", name="qb")
                nc.vector.tensor_copy(out=qb, in_=d["qv"].rearrange("p c h w -> p c (h w)"))
                kb = sm.tile([128, 2, 64], BF16, tag="kb", name="kb")
                nc.vector.tensor_copy(out=kb, in_=d["kv"].rearrange("p c h w -> p c (h w)"))
                qb0 = sm.tile([32, 8, 64], BF16, tag="qb0", name="qb0")
                kb0 = sm.tile([32, 8, 64], BF16, tag="kb0", name="kb0")
                for h in range(8):
                    ch, off = h // 4, (h % 4) * 32
                    nc.vector.tensor_copy(out=qb0[:, h, :], in_=qb[off:off + 32, ch, :])
                    nc.vector.tensor_copy(out=kb0[:, h, :], in_=kb[off:off + 32, ch, :])
                pqk = pps.tile([64, 8, 64], F32, tag="pa", bufs=1, name="pqk")
                for h in range(8):
                    nc.tensor.matmul(pqk[:, h, :], kb0[:, h, :], qb0[:, h, :],
                                     start=(h == 0), stop=(h == 7))
                a2t = sm.tile([64, 8, 64], FP8, tag="a2t", name="a2t")
                nc.scalar.copy(out=a2t, in_=pqk)
                pys = pps.tile([64, 8, 33], F32, tag="pa", bufs=1, name="pys")
                for t in range(NT):
                    K = 128 if t < NT - 1 else 64
                    pe = pps.tile([128, 512], F32, tag="pgB", bufs=2, name="pe")
                    nc.tensor.matmul(pe[:K, :], G[:, t * 128:t * 128 + K],
                                     a2t.rearrange("p h q -> p (h q)"),
                                     start=True, stop=True)
                    eT = tmp.tile([128, 512], BF16, tag="eT", bufs=2, name="eT")
                    nc.scalar.activation(out=eT[:K], in_=pe[:K], func=AF.Exp,
                                         scale=float(DIM) ** -0.5)
                    for h in range(8):
                        nc.tensor.matmul(pys[:, h, :], eT[:K, h * 64:(h + 1) * 64],
                                         v_aug[:K, t, h, :],
                                         start=(t == 0 and h == 0),
                                         stop=(t == NT - 1 and h == 7))
                rz = sm.tile([64, 8], F32, tag="rz", name="rz")
                nc.vector.reciprocal(out=rz, in_=pys[:, :, 32])
                y_rT = sm.tile([64, 256], FP8, tag="y_rT", name="y_rT")
                for h in range(8):
                    nc.scalar.activation(out=y_rT[:, h * 32:(h + 1) * 32],
                                         in_=pys[:, h, 0:32], func=AF.Copy,
                                         scale=rz[:, h:h + 1])
                for ch in range(2):
                    for s in range(7):
                        pu = pps.tile([128, 448], F32, tag="pgB", bufs=2, name="pu")
                        nc.tensor.matmul(pu, y_rT[:, ch * 128:(ch + 1) * 128],
                                         G[:, s * 448:(s + 1) * 448], start=True, stop=True)
                        nc.vector.tensor_copy(
                            out=yu_pad[:, ch, 1 + 8 * s:9 + 8 * s, 1:57], in_=pu)
            def st_vu(b):
                d = S[b]
                ysum = big.tile([128, 2, N], FP8, tag="ysum", name=f"ysum{b}")
                for ch in range(2):
                    def drainv(s, r0, cp, ch=ch):
                        nc.vector.scalar_tensor_tensor(
                            out=ysum[:, ch, r0 * RES:(r0 + 8) * RES],
                            in0=cp.rearrange("p a b -> p (a b)"),
                            scalar=col(14 + ch),
                            in1=d["skip2"][:, ch, r0 * RES:(r0 + 8) * RES],
                            op0=OP.mult, op1=OP.add)
                    conv(yu_pad[:, ch], 4 + ch, 8, drainv)
                d.update(ysum=ysum)

            def st_p(b):
                d = S[b]
                p_ct = big.tile([128, 2, NP], BF16, tag="shB", name=f"p_ct{b}")
                for mc in range(2):
                    for s in range(7):
                        pp = pps.tile([128, 448], F32, tag="pgB", bufs=2, name="pp")
                        nc.tensor.matmul(pp, Wp[:, :, mc, :],
                                         d["ysum"][:, :, s * 448:(s + 1) * 448],
                                         start=True, stop=True, perf_mode=PM)
                        nc.vector.tensor_scalar(
                            out=p_ct[:, mc, s * 448:(s + 1) * 448], in0=pp,
                            scalar1=1.0 / SW, scalar2=col(16 + mc),
                            op0=OP.mult, op1=OP.add)
                p_tok = big.tile([128, 2, NT, 128], BF16, tag="v_tok", name=f"p_tok{b}")
                for kc in range(2):
                    nc.sync.dma_start_transpose(out=p_tok[:, kc], in_=p_ct[:, kc])
                for t in range(NT):
                    nc.vector.tensor_tensor(
                        out=d["x_tok"][:, t, :], in0=d["x_tok"][:, t, :],
                        in1=p_tok[:, :, t, :], op=OP.add)

            def st_ln2(b):
                d = S[b]
                xn2_tok = big.tile([128, NT, 256], BF16, tag="xn_tok", bufs=2,
                                   name=f"xn2_tok{b}")
                xn2_cti = big.tile([128, NT * 256], BF16, tag="shA", name=f"xn2_cti{b}")
                xn2_fp8 = big.tile([128, 2, N], FP8, tag="shC", name=f"xn2_fp8{b}")
                ln_to_ct(d["x_tok"], xn2_tok, xn2_cti, xn2_fp8)
                d.update(xn2_fp8=xn2_fp8)

            def st_ffn(b, half):
                d = S[b]
                if half == 0:
                    d["z2"] = big.tile([128, 8, N], FP8, tag="shZ", name=f"z2{b}")
                z2 = d["z2"]
                for m4 in range(4):
                    mc = half * 4 + m4
                    for s in range(7):
                        pf = pps.tile([128, 448], F32, tag="pgA", bufs=2, name="pf")
                        nc.tensor.matmul(pf, F1[:, :, mc, :],
                                         d["xn2_fp8"][:, :, s * 448:(s + 1) * 448],
                                         start=True, stop=True, perf_mode=PM)
                        nc.scalar.activation(
                            out=z1_pad[:, m4, 1 + 8 * s:9 + 8 * s, 1:57],
                            in_=pf, func=AF.Gelu, scale=colff[:, 0, mc:mc + 1],
                            bias=colff[:, 1, mc:mc + 1])
                for m4 in range(4):
                    hc = half * 4 + m4
                    def drainf(s, r0, cp, hc=hc):
                        nc.scalar.activation(
                            out=z2[:, hc, r0 * RES:(r0 + 8) * RES],
                            in_=cp.rearrange("p a b -> p (a b)"), func=AF.Gelu,
                            scale=colff[:, 2, hc:hc + 1], bias=colff[:, 3, hc:hc + 1])
                    conv(z1_pad[:, m4], 6 + hc, 8, drainf)

            def st_f3_out(b):
                d = S[b]
                z3_ct = big.tile([128, 2, NP], BF16, tag="shB", name=f"z3_ct{b}")
                for mc in range(2):
                    for s in range(7):
                        pf3 = pps.tile([128, 448], F32, tag="pgB", bufs=2, name="pf3")
                        for p in range(4):
                            nc.tensor.matmul(pf3, F3[:, p, :, mc, :],
                                             d["z2"][:, 2 * p:2 * p + 2,
                                                     s * 448:(s + 1) * 448],
                                             start=(p == 0), stop=(p == 3), perf_mode=PM)
                        nc.vector.tensor_scalar(
                            out=z3_ct[:, mc, s * 448:(s + 1) * 448], in0=pf3,
                            scalar1=col(18 + mc), scalar2=col(20 + mc),
                            op0=OP.mult, op1=OP.add)
                z3_tok = big.tile([128, 2, NT, 128], BF16, tag="v_tok", name=f"z3_tok{b}")
                for kc in range(2):
                    nc.sync.dma_start_transpose(out=z3_tok[:, kc], in_=z3_ct[:, kc])
                    for t in range(NT):
                        rows = 128 if t < NT - 1 else 64
                        ot = tmp.tile([128, 128], F32, tag="ot", bufs=4, name="ot")
                        nc.vector.tensor_tensor(
                            out=ot[:rows], in0=d["x_tok"][:rows, t,
                                                          kc * 128:(kc + 1) * 128],
                            in1=z3_tok[:rows, kc, t, :], op=OP.add)
                        nc.sync.dma_start(
                            out=out[b, t * 128:t * 128 + rows,
                                    kc * 128:(kc + 1) * 128],
                            in_=ot[:rows])

            # software-pipelined emission across the 2 batch elements
            marks = _CACHE.setdefault("marks", [])
            def mark(nm):
                n = int(nc.get_next_instruction_name()[2:])
                marks.append((nm, n))
            mark("start")
            st_load_ln1(0); mark("ln1.0")
            st_v_convs(0); mark("vconv.0")
            st_load_ln1(1); mark("ln1.1")
            st_attn(0); mark("attn.0")
            st_vu(0); mark("vu.0")
            st_p(0); mark("p.0")
            st_v_convs(1); mark("vconv.1")
            st_attn(1); mark("attn.1")
            st_ln2(0); mark("ln2.0")
            st_vu(1); mark("vu.1")
            st_p(1); mark("p.1")
            st_ffn(0, 0); mark("ffn0.0")
            st_ln2(1); mark("ln2.1")
            st_ffn(0, 1); mark("ffn1.0")
            st_f3_out(0); mark("f3.0")
            st_ffn(1, 0); mark("ffn0.1")
            st_ffn(1, 1); mark("ffn1.1")
            st_f3_out(1); mark("f3.1")

    nc.compile()
    names = dict(x=xg.name, out=out.name, G=G_d.name, Wv=Wv_d.name, Wp=Wp_d.name,
                 F1=F1_d.name, F3=F3_d.name, dgp=dgp_d.name, dgs=dgs_d.name,
                 cols=cols_d.name, colff=colff_d.name)
    return nc, names


def _upmat():
    def idx(n, s):
        src = np.maximum((np.arange(n * s) + 0.5) / s - 0.5, 0.0)
        i0 = np.minimum(np.floor(src).astype(np.int64), n - 1)
        i1 = np.minimum(i0 + 1, n - 1)
        return i0, i1, src - i0
    R = np.zeros((RES, SR + 1), np.float64)
    i0, i1, t = idx(SR + 1, SR)
    for y in range(RES):
        R[y, i0[y]] += 1 - t[y]
        R[y, i1[y]] += t[y]
    return np.einsum('yi,xj->ijyx', R, R).reshape(64, N).astype(np.float32)


def _cols(v):
    return np.ascontiguousarray(np.asarray(v, np.float32).reshape(-1, 128).T)


def _dr_w(w):
    """W [M,K] -> DR lhsT [128, K/256 pairs..., M/128, 128] as [128, 2, M/128, 128]
    per 256-K block; returns [128, nkp, 2, nm, 128]? here K=256 -> [128, 2, nm, 128]."""
    M, K = w.shape
    nm = M // 128
    wT = w.T.reshape(2, 128, nm, 128)          # [kc, k, mc, m]
    return np.ascontiguousarray(wT.transpose(1, 0, 2, 3)).astype(F8)


def kernel(**inputs):
    if "prog" not in _CACHE:
        _CACHE["prog"] = _build()
    nc, nm = _CACHE["prog"]
    ii = {k: np.asarray(v) for k, v in inputs.items()}

    inv1 = ii["bn1_g"] / np.sqrt(ii["bn1_v"] + BN_EPS)
    inv2 = ii["bn2_g"] / np.sqrt(ii["bn2_v"] + BN_EPS)
    invv = ii["bnv_g"] / np.sqrt(ii["bnv_v"] + BN_EPS)
    invf1 = ii["bf1_g"] / np.sqrt(ii["bf1_v"] + BN_EPS)
    invf2 = ii["bf2_g"] / np.sqrt(ii["bf2_v"] + BN_EPS)
    invf3 = ii["bf3_g"] / np.sqrt(ii["bf3_v"] + BN_EPS)

    g1, b1 = ii["n1_g"], ii["n1_b"]
    g2, b2 = ii["n2_g"], ii["n2_b"]

    Wv_eff = ii["Wv"] * g1[None, :]
    B_v = ii["Wv"] @ b1
    F1_eff = ii["f1_w"] * g2[None, :]
    betaf1 = invf1 * (ii["f1_w"] @ b2 + ii["f1_b"] - ii["bf1_m"]) + ii["bf1_b"]
    B_p = ii["Wp"] @ (ii["bnv_b"] - ii["bnv_m"] * invv) + ii["bp"]
    betaf3 = invf3 * (ii["f3_b"] - ii["bf3_m"]) + ii["bf3_b"]

    # conv diagonals (raw weights * SW; bn inv applied at drain scale)
    dgp = np.zeros((128, 14, 3, 2, 128), F8)
    dgs = np.zeros((128, 14, 3, 128), F8)
    convw = [ii["c1_w"][:, 0], ii["c2_w"][:, 0], ii["vu_w"][:, 0], ii["f2_w"][:, 0]]
    ci = 0
    ar = np.arange(128)
    for w in convw:
        nch = w.shape[0] // 128
        wr = w.reshape(nch, 128, 3, 3)
        for c in range(nch):
            for x in range(3):
                dgp[ar, ci, x, 0, ar] = (wr[c, :, 0, x] * SW).astype(F8)
                dgp[ar, ci, x, 1, ar] = (wr[c, :, 1, x] * SW).astype(F8)
                dgs[ar, ci, x, ar] = (wr[c, :, 2, x] * SW).astype(F8)
            ci += 1

    beta1 = ii["bn1_b"] - ii["bn1_m"] * inv1
    beta2 = ii["bn2_b"] - ii["bn2_m"] * inv2

    cols = np.zeros((128, 24), np.float32)
    cols[:, 0:2] = _cols(B_v)
    cols[:, 2:4] = _cols(inv1 / SW)
    cols[:, 4:6] = _cols(beta1)
    cols[:, 6:8] = _cols(inv1 / (49.0 * SW))
    cols[:, 8:10] = _cols(inv2 / SW)
    cols[:, 10:12] = _cols(beta2)
    cols[:, 12:14] = _cols(inv2 / (49.0 * SW))
    cols[:, 14:16] = _cols(invv / SW)
    cols[:, 16:18] = _cols(B_p)
    cols[:, 18:20] = _cols(invf3 / SW)
    cols[:, 20:22] = _cols(betaf3)
    # 22/23 unused placeholders (f2 uses colf2 via cols? see colf2 below)

    betaf2 = invf2 * (ii["f2_b"] - ii["bf2_m"]) + ii["bf2_b"]
    colff = np.zeros((128, 4, 8), np.float32)
    colff[:, 0, :] = _cols(invf1 / SW)
    colff[:, 1, :] = _cols(betaf1)
    colff[:, 2, :] = _cols(invf2 / SW)
    colff[:, 3, :] = _cols(betaf2)

    consts = {
        nm["G"]: _upmat().astype(F8),
        nm["Wv"]: _dr_w(Wv_eff * SW),
        nm["Wp"]: _dr_w(ii["Wp"] * SW),
        nm["F1"]: _dr_w(F1_eff * SW),
        nm["dgp"]: dgp, nm["dgs"]: dgs,
        nm["cols"]: cols, nm["colff"]: colff,
    }
    # F3: [128 k, pair p, kc-in-pair, mc, 128 m]
    f3T = (ii["f3_w"] * SW).T.reshape(4, 2, 128, 2, 128)   # [p, kc, k, mc, m]
    consts[nm["F3"]] = np.ascontiguousarray(f3T.transpose(2, 0, 1, 3, 4)).astype(F8)

    x = np.ascontiguousarray(ii["x"].astype(BF))
    in_maps = [dict(consts, **{nm["x"]: np.ascontiguousarray(x[c * BPC:(c + 1) * BPC])})
               for c in range(NCORES)]
    kw = {}
    if _CACHE.get("trace"):
        import shutil
        shutil.rmtree("/tmp/bass_trace", ignore_errors=True)
        os.makedirs("/tmp/bass_trace", exist_ok=True)
        kw = dict(trace=True, trace_cores=[0], tmpdir="/tmp/bass_trace")
    res = run_bass_kernel_spmd(nc, in_maps, list(range(NCORES)), **kw)
    _CACHE["last_res"] = res
    return np.concatenate([res.results[c][nm["out"]] for c in range(NCORES)], axis=0)



# revision 23
# speedup vs baseline: 1.0426x; 1.0426x over previous
import sys, os
sys.path.insert(0, '/opt/trn_rl_repo')
import numpy as np
import ml_dtypes

import concourse.bass as bass
import concourse.mybir as mybir
import concourse.tile as tile
from concourse import bacc
from concourse.bass_utils import run_bass_kernel_spmd

F32 = mybir.dt.float32
BF16 = mybir.dt.bfloat16
FP8 = mybir.dt.float8e4
PM = mybir.MatmulPerfMode.DoubleRow
AF = mybir.ActivationFunctionType
OP = mybir.AluOpType
F8 = ml_dtypes.float8_e4m3fn
BF = ml_dtypes.bfloat16

B, DIM, HEADS, SR, RES, HID = 16, 256, 8, 7, 56, 1024
N = RES * RES              # 3136
NP = 3200                  # padded token count (25*128) for DMA transpose
LN_EPS, BN_EPS = 1e-6, 1e-5
NCORES = 8
BPC = B // NCORES
NT = 25
SW = 16.0                  # fp8 weight rescale

_CACHE = {}


def _build():
    nc = bacc.Bacc(None, target_bir_lowering=False, debug=True)

    xg = nc.dram_tensor([BPC, N, DIM], BF16, kind="ExternalInput")
    out = nc.dram_tensor([BPC, N, DIM], BF16, kind="ExternalOutput")
    G_d = nc.dram_tensor([128, N], FP8, kind="ExternalInput")
    Wv_d = nc.dram_tensor([128, 2, 2, 128], FP8, kind="ExternalInput")
    Wp_d = nc.dram_tensor([128, 2, 2, 128], FP8, kind="ExternalInput")
    F1_d = nc.dram_tensor([128, 2, 8, 128], FP8, kind="ExternalInput")
    F3_d = nc.dram_tensor([128, 4, 2, 2, 128], FP8, kind="ExternalInput")
    dgp_d = nc.dram_tensor([128, 14, 3, 2, 128], FP8, kind="ExternalInput")
    dgs_d = nc.dram_tensor([128, 14, 3, 128], FP8, kind="ExternalInput")
    # per-partition columns: ln g/b not needed (folded); drain scales/biases
    cols_d = nc.dram_tensor([128, 24], F32, kind="ExternalInput")
    colff_d = nc.dram_tensor([128, 4, 8], F32, kind="ExternalInput")

    # cols layout (k index):
    # 0,1: B_v ; 2,3: inv1/S ; 4,5: beta1 ; 6,7: invq1 = inv1/(49 S) ;
    # 8,9: inv2/S ; 10,11: beta2 ; 12,13: invq2 ; 14,15: invv/S ;
    # 16,17: B_p ; 18,19: invf3/S ; 20,21: betaf3 ; 22: unused, 23: unused
    # colf1: [:, 0, :]=invf1/S  [:, 1, :]=betaf1'

    with tile.TileContext(nc) as tc:
        with (
            tc.tile_pool(name="cst", bufs=1) as cst,
            tc.tile_pool(name="big", bufs=1) as big,
            tc.tile_pool(name="sm", bufs=1) as sm,
            tc.tile_pool(name="tmp", bufs=3) as tmp,
            tc.tile_pool(name="pps", bufs=8, space="PSUM") as pps,
        ):
            G = cst.tile([128, N], FP8)
            nc.sync.dma_start(out=G, in_=G_d[:])
            Wv = cst.tile([128, 2, 2, 128], FP8)
            nc.sync.dma_start(out=Wv, in_=Wv_d[:])
            Wp = cst.tile([128, 2, 2, 128], FP8)
            nc.sync.dma_start(out=Wp, in_=Wp_d[:])
            F1 = cst.tile([128, 2, 8, 128], FP8)
            nc.sync.dma_start(out=F1, in_=F1_d[:])
            F3 = cst.tile([128, 4, 2, 2, 128], FP8)
            nc.sync.dma_start(out=F3, in_=F3_d[:])
            dgp = cst.tile([128, 14, 3, 2, 128], FP8)
            nc.sync.dma_start(out=dgp, in_=dgp_d[:])
            dgs = cst.tile([128, 14, 3, 128], FP8)
            nc.sync.dma_start(out=dgs, in_=dgs_d[:])
            cols = cst.tile([128, 24], F32)
            nc.sync.dma_start(out=cols, in_=cols_d[:])
            colff = cst.tile([128, 4, 8], F32)
            nc.sync.dma_start(out=colff, in_=colff_d[:])
            epsln = cst.tile([128, 1], F32)
            nc.vector.memset(epsln, LN_EPS)

            # persistent padded conv-input buffers (borders stay zero)
            v_pad = cst.tile([128, 2, 58, 64], FP8)
            nc.vector.memset(v_pad.bitcast(mybir.dt.int32), 0)
            s1_pad = cst.tile([128, 2, 58, 64], FP8)
            nc.vector.memset(s1_pad.bitcast(mybir.dt.int32), 0)
            yu_pad = cst.tile([128, 2, 58, 64], FP8)
            nc.vector.memset(yu_pad.bitcast(mybir.dt.int32), 0)
            z1_pad = cst.tile([128, 4, 58, 64], FP8)
            nc.vector.memset(z1_pad.bitcast(mybir.dt.int32), 0)
            v_aug = cst.tile([128, NT, 4, 66], BF16)
            nc.vector.memset(v_aug[:, :, :, 32:33], 1.0)
            nc.vector.memset(v_aug[:, :, :, 65:66], 1.0)

            def col(k):
                return cols[:, k:k + 1]

            def ln_to_ct(x_tok, xn_tok, xn_cti, xn_fp8):
                """token-major LN stats+normalize -> bf16 xn_tok; DMA-T -> interleaved
                bf16 xn_cti; scalar cast -> plain fp8 [128,2,NP]."""
                for t in range(NT):
                    rows = 128 if t < NT - 1 else 64
                    st = tmp.tile([128, 6], F32, tag="st", bufs=4, name="st")
                    nc.vector.bn_stats(out=st[:rows], in_=x_tok[:rows, t, :])
                    mv = tmp.tile([128, 2], F32, tag="mv", bufs=4, name="mv")
                    nc.vector.bn_aggr(out=mv[:rows], in_=st[:rows])
                    sd = tmp.tile([128, 1], F32, tag="sd", bufs=4, name="sd")
                    nc.scalar.activation(out=sd[:rows], in_=mv[:rows, 1:2],
                                         func=AF.Sqrt, bias=epsln[:rows])
                    rs = tmp.tile([128, 1], F32, tag="rs", bufs=4, name="rs")
                    nc.vector.reciprocal(out=rs[:rows], in_=sd[:rows])
                    nc.vector.tensor_scalar(out=xn_tok[:rows, t, :], in0=x_tok[:rows, t, :],
                                            scalar1=mv[:rows, 0:1], scalar2=rs[:rows],
                                            op0=OP.subtract, op1=OP.mult)
                srcv = xn_cti.rearrange("p (t k j) -> p t k j", k=2, j=128)
                for q0, q1 in ((0, 7), (7, 13), (13, 19), (19, 25)):
                    nc.sync.dma_start_transpose(
                        out=xn_cti[:, q0 * 256:q1 * 256].rearrange(
                            "p (m j) -> p m j", j=128),
                        in_=xn_tok[:, q0:q1, :].rearrange("p t c -> p (t c)"))
                    tend = min(q1, 24)
                    nc.scalar.copy(
                        out=xn_fp8[:, :, q0 * 128:tend * 128].rearrange(
                            "p k (t j) -> p k t j", j=128),
                        in_=srcv[:, q0:tend].rearrange("p t k j -> p k t j"))
                nc.scalar.copy(out=xn_fp8[:, :, 3072:3136], in_=srcv[:, 24, :, 0:64])

            def conv(pad_ch, ci, nrows, drain):
                """fp8-DR conv on padded [58,64] image chunk. pairs ((0,x),(1,x)) + singles
                (2,x). nrows: stripe rows (7 for pooled convs, 8 otherwise)."""
                nstripe = RES // nrows
                for s in range(nstripe):
                    r0 = s * nrows
                    cp = pps.tile([128, nrows, RES], F32, tag="cp", bufs=3,
                                  name="cp")
                    for x in range(3):
                        base = pad_ch[:, r0:r0 + nrows, x:x + RES]
                        rhs = base.copy()
                        rhs.ap.insert(1, (64, 2))
                        nc.tensor.matmul(cp, dgp[:, ci, x, :, :], rhs,
                                         start=(x == 0), stop=False, perf_mode=PM)
                    for x in range(3):
                        base = pad_ch[:, r0 + 2:r0 + 2 + nrows, x:x + RES]
                        nc.tensor.matmul(cp, dgs[:, ci, x, :], base,
                                         start=False, stop=(x == 2))
                    drain(s, r0, cp)

            def pool_from(cp, pool_out, ch, s, sc_k):
                t2 = tmp.tile([128, 8], F32, tag="t2", bufs=4, name="t2")
                nc.vector.tensor_reduce(
                    out=t2, in_=cp.rearrange("p h (wb k) -> p wb h k", k=7),
                    axis=mybir.AxisListType.XY, op=OP.add)
                nc.vector.tensor_scalar(out=pool_out[:, ch, s, :], in0=t2,
                                        scalar1=col(sc_k + ch), scalar2=col(sc_k - 2 + ch),
                                        op0=OP.mult, op1=OP.add)

            S = [dict() for _ in range(BPC)]

            def st_load_ln1(b):
                d = S[b]
                x_tok = big.tile([128, NT, 256], BF16, tag="x_tok", bufs=2,
                                 name=f"x_tok{b}")
                nc.sync.dma_start(
                    out=x_tok[:, 0:24, :],
                    in_=xg[b, 0:3072, :].rearrange("(t p) c -> p t c", p=128))
                nc.sync.dma_start(out=x_tok[:64, 24, :], in_=xg[b, 3072:3136, :])
                xn_tok = big.tile([128, NT, 256], BF16, tag="xn_tok", bufs=2,
                                  name=f"xn_tok{b}")
                xn_cti = big.tile([128, NT * 256], BF16, tag="shA", name=f"xn_cti{b}")
                xn_fp8 = big.tile([128, 2, N], FP8, tag="shC", name=f"xn_fp8{b}")
                ln_to_ct(x_tok, xn_tok, xn_cti, xn_fp8)
                d.update(x_tok=x_tok, xn_fp8=xn_fp8)

            def st_v_convs(b):
                d = S[b]
                v_dense = big.tile([128, 2, NP], BF16, tag="shA", name=f"v_dense{b}")
                for mc in range(2):
                    for s in range(7):
                        pv = pps.tile([128, 448], F32, tag="pgA", bufs=2, name="pv")
                        nc.tensor.matmul(pv, Wv[:, :, mc, :],
                                         d["xn_fp8"][:, :, s * 448:(s + 1) * 448],
                                         start=True, stop=True, perf_mode=PM)
                        nc.vector.tensor_scalar(
                            out=v_dense[:, mc, s * 448:(s + 1) * 448], in0=pv,
                            scalar1=1.0 / SW, scalar2=col(0 + mc),
                            op0=OP.mult, op1=OP.add)
                        nc.vector.tensor_scalar(
                            out=v_pad[:, mc, 1 + 8 * s:9 + 8 * s, 1:57], in0=pv,
                            scalar1=1.0 / SW, scalar2=col(0 + mc),
                            op0=OP.mult, op1=OP.add)
                qv = sm.tile([128, 2, 8, 8], F32, tag="qv", name=f"qv{b}")
                for ch in range(2):
                    def drain1(s, r0, cp, ch=ch):
                        pool_from(cp, qv, ch, s, 6)
                        nc.scalar.activation(
                            out=s1_pad[:, ch, 1 + r0:1 + r0 + 7, 1:57],
                            in_=cp, func=AF.Gelu, scale=col(2 + ch), bias=col(4 + ch))
                    conv(v_pad[:, ch], 0 + ch, 7, drain1)
                kv = sm.tile([128, 2, 8, 8], F32, tag="kv", name=f"kv{b}")
                skip2 = big.tile([128, 2, NP], BF16, tag="xn_tok", bufs=2,
                                 name=f"skip2{b}")
                for ch in range(2):
                    def drain2(s, r0, cp, ch=ch):
                        pool_from(cp, kv, ch, s, 12)
                        nc.scalar.activation(
                            out=skip2[:, ch, r0 * RES:(r0 + 7) * RES],
                            in_=cp, func=AF.Gelu, scale=col(8 + ch), bias=col(10 + ch))
                    conv(s1_pad[:, ch], 2 + ch, 7, drain2)
                d.update(v_dense=v_dense, qv=qv, kv=kv, skip2=skip2)

            def st_attn(b):
                d = S[b]
                v_tok = big.tile([128, 2, NT, 128], BF16, tag="v_tok", name=f"v_tok{b}")
                for kc in range(2):
                    nc.sync.dma_start_transpose(out=v_tok[:, kc], in_=d["v_dense"][:, kc])
                for kc in range(2):
                    nc.vector.tensor_copy(
                        out=v_aug[:, :, 2 * kc:2 * kc + 2, :].rearrange(
                            "p t i (s d) -> p t i s d", s=2)[:, :, :, :, 0:32],
                        in_=v_tok[:, kc].rearrange("p t (i s d) -> p t i s d",
                                                   i=2, s=2))
                qb = sm.tile([128, 2, 64], BF16, tag="qb", name="qb")
                nc.vector.tensor_copy(out=qb, in_=d["qv"].rearrange("p c h w -> p c (h w)"))
                kb = sm.tile([128, 2, 64], BF16, tag="kb", name="kb")
                nc.vector.tensor_copy(out=kb, in_=d["kv"].rearrange("p c h w -> p c (h w)"))
                qb0 = sm.tile([32, 8, 64], BF16, tag="qb0", name="qb0")
                kb0 = sm.tile([32, 8, 64], BF16, tag="kb0", name="kb0")
                for h in range(8):
                    ch, off = h // 4, (h % 4) * 32
                    nc.vector.tensor_copy(out=qb0[:, h, :], in_=qb[off:off + 32, ch, :])
                    nc.vector.tensor_copy(out=kb0[:, h, :], in_=kb[off:off + 32, ch, :])
                pqk = pps.tile([64, 8, 64], F32, tag="pa", bufs=1, name="pqk")
                for h in range(8):
                    nc.tensor.matmul(pqk[:, h, :], kb0[:, h, :], qb0[:, h, :],
                                     start=(h == 0), stop=(h == 7))
                a2t = sm.tile([64, 8, 64], FP8, tag="a2t", name="a2t")
                nc.scalar.copy(out=a2t, in_=pqk)
                pys = pps.tile([128, 4, 66], F32, tag="pa", bufs=1, name="pys")
                for t in range(NT):
                    K = 128 if t < NT - 1 else 64
                    pe = pps.tile([128, 512], F32, tag="pgB", bufs=2, name="pe")
                    nc.tensor.matmul(pe[:K, :], G[0:64, t * 128:t * 128 + K],
                                     a2t.rearrange("p h q -> p (h q)"),
                                     start=True, stop=True)
                    eT = tmp.tile([128, 512], BF16, tag="eT", bufs=2, name="eT")
                    nc.scalar.activation(out=eT[:K], in_=pe[:K], func=AF.Exp,
                                         scale=float(DIM) ** -0.5)
                    for i in range(4):
                        nc.tensor.matmul(pys[:, i, :], eT[:K, i * 128:(i + 1) * 128],
                                         v_aug[:K, t, i, :],
                                         start=(t == 0 and i == 0),
                                         stop=(t == NT - 1 and i == 3))
                rz = sm.tile([128, 4], F32, tag="rz", name="rz")
                nc.vector.reciprocal(out=rz[0:64], in_=pys[0:64, :, 32])
                nc.vector.reciprocal(out=rz[64:128], in_=pys[64:128, :, 65])
                y_rT = sm.tile([64, 256], FP8, tag="y_rT", name="y_rT")
                yv = y_rT.rearrange("p (i x d) -> p i x d", i=4, x=2)
                for i in range(4):
                    nc.vector.tensor_scalar(
                        out=yv[:, i, 0, :], in0=pys[0:64, i, 0:32],
                        scalar1=rz[0:64, i:i + 1], scalar2=None, op0=OP.mult)
                y_odd = sm.tile([128, 4, 32], FP8, tag="y_odd", name="y_odd")
                for i in range(4):
                    nc.vector.tensor_scalar(
                        out=y_odd[64:128, i, :], in0=pys[64:128, i, 33:65],
                        scalar1=rz[64:128, i:i + 1], scalar2=None, op0=OP.mult)
                nc.sync.dma_start(out=yv[:, :, 1, :], in_=y_odd[64:128])
                for ch in range(2):
                    for s in range(7):
                        pu = pps.tile([128, 448], F32, tag="pgB", bufs=2, name="pu")
                        nc.tensor.matmul(pu, y_rT[:, ch * 128:(ch + 1) * 128],
                                         G[0:64, s * 448:(s + 1) * 448],
                                         start=True, stop=True)
                        nc.vector.tensor_copy(
                            out=yu_pad[:, ch, 1 + 8 * s:9 + 8 * s, 1:57], in_=pu)
            def st_vu(b):
                d = S[b]
                ysum = big.tile([128, 2, N], FP8, tag="ysum", name=f"ysum{b}")
                for ch in range(2):
                    def drainv(s, r0, cp, ch=ch):
                        nc.vector.scalar_tensor_tensor(
                            out=ysum[:, ch, r0 * RES:(r0 + 8) * RES],
                            in0=cp.rearrange("p a b -> p (a b)"),
                            scalar=col(14 + ch),
                            in1=d["skip2"][:, ch, r0 * RES:(r0 + 8) * RES],
                            op0=OP.mult, op1=OP.add)
                    conv(yu_pad[:, ch], 4 + ch, 8, drainv)
                d.update(ysum=ysum)

            def st_p(b):
                d = S[b]
                p_ct = big.tile([128, 2, NP], BF16, tag="shB", name=f"p_ct{b}")
                for mc in range(2):
                    for s in range(7):
                        pp = pps.tile([128, 448], F32, tag="pgB", bufs=2, name="pp")
                        nc.tensor.matmul(pp, Wp[:, :, mc, :],
                                         d["ysum"][:, :, s * 448:(s + 1) * 448],
                                         start=True, stop=True, perf_mode=PM)
                        nc.vector.tensor_scalar(
                            out=p_ct[:, mc, s * 448:(s + 1) * 448], in0=pp,
                            scalar1=1.0 / SW, scalar2=col(16 + mc),
                            op0=OP.mult, op1=OP.add)
                p_tok = big.tile([128, 2, NT, 128], BF16, tag="v_tok", name=f"p_tok{b}")
                for kc in range(2):
                    nc.sync.dma_start_transpose(out=p_tok[:, kc], in_=p_ct[:, kc])
                    nc.vector.tensor_tensor(
                        out=d["x_tok"][:, :, kc * 128:(kc + 1) * 128],
                        in0=d["x_tok"][:, :, kc * 128:(kc + 1) * 128],
                        in1=p_tok[:, kc], op=OP.add)

            def st_ln2(b):
                d = S[b]
                xn2_tok = big.tile([128, NT, 256], BF16, tag="xn_tok", bufs=2,
                                   name=f"xn2_tok{b}")
                xn2_cti = big.tile([128, NT * 256], BF16, tag="shA", name=f"xn2_cti{b}")
                xn2_fp8 = big.tile([128, 2, N], FP8, tag="shC", name=f"xn2_fp8{b}")
                ln_to_ct(d["x_tok"], xn2_tok, xn2_cti, xn2_fp8)
                d.update(xn2_fp8=xn2_fp8)

            def st_ffn(b, half):
                d = S[b]
                if half == 0:
                    d["z2"] = big.tile([128, 8, N], FP8, tag="shZ", name=f"z2{b}")
                z2 = d["z2"]
                for m4 in range(4):
                    mc = half * 4 + m4
                    for s in range(7):
                        pf = pps.tile([128, 448], F32, tag="pgA", bufs=2, name="pf")
                        nc.tensor.matmul(pf, F1[:, :, mc, :],
                                         d["xn2_fp8"][:, :, s * 448:(s + 1) * 448],
                                         start=True, stop=True, perf_mode=PM)
                        nc.scalar.activation(
                            out=z1_pad[:, m4, 1 + 8 * s:9 + 8 * s, 1:57],
                            in_=pf, func=AF.Gelu, scale=colff[:, 0, mc:mc + 1],
                            bias=colff[:, 1, mc:mc + 1])
                for m4 in range(4):
                    hc = half * 4 + m4
                    def drainf(s, r0, cp, hc=hc):
                        nc.scalar.activation(
                            out=z2[:, hc, r0 * RES:(r0 + 8) * RES],
                            in_=cp.rearrange("p a b -> p (a b)"), func=AF.Gelu,
                            scale=colff[:, 2, hc:hc + 1], bias=colff[:, 3, hc:hc + 1])
                    conv(z1_pad[:, m4], 6 + hc, 8, drainf)

            def st_f3_out(b):
                d = S[b]
                z3_ct = big.tile([128, 2, NP], BF16, tag="shB", name=f"z3_ct{b}")
                for mc in range(2):
                    for s in range(7):
                        pf3 = pps.tile([128, 448], F32, tag="pgB", bufs=2, name="pf3")
                        for p in range(4):
                            nc.tensor.matmul(pf3, F3[:, p, :, mc, :],
                                             d["z2"][:, 2 * p:2 * p + 2,
                                                     s * 448:(s + 1) * 448],
                                             start=(p == 0), stop=(p == 3), perf_mode=PM)
                        nc.vector.tensor_scalar(
                            out=z3_ct[:, mc, s * 448:(s + 1) * 448], in0=pf3,
                            scalar1=col(18 + mc), scalar2=col(20 + mc),
                            op0=OP.mult, op1=OP.add)
                z3_tok = big.tile([128, 2, NT, 128], BF16, tag="v_tok", name=f"z3_tok{b}")
                for kc in range(2):
                    nc.sync.dma_start_transpose(out=z3_tok[:, kc], in_=z3_ct[:, kc])
                    nc.vector.tensor_tensor(
                        out=d["x_tok"][:, :, kc * 128:(kc + 1) * 128],
                        in0=d["x_tok"][:, :, kc * 128:(kc + 1) * 128],
                        in1=z3_tok[:, kc], op=OP.add)
                nc.sync.dma_start(
                    out=out[b, 0:3072, :].rearrange("(t p) c -> p t c", p=128),
                    in_=d["x_tok"][:, 0:24, :])
                nc.sync.dma_start(out=out[b, 3072:3136, :], in_=d["x_tok"][:64, 24, :])

            # software-pipelined emission across the 2 batch elements
            marks = _CACHE.setdefault("marks", [])
            def mark(nm):
                n = int(nc.get_next_instruction_name()[2:])
                marks.append((nm, n))
            mark("start")
            st_load_ln1(0); mark("ln1.0")
            st_v_convs(0); mark("vconv.0")
            st_load_ln1(1); mark("ln1.1")
            st_attn(0); mark("attn.0")
            st_vu(0); mark("vu.0")
            st_p(0); mark("p.0")
            st_v_convs(1); mark("vconv.1")
            st_attn(1); mark("attn.1")
            st_ln2(0); mark("ln2.0")
            st_vu(1); mark("vu.1")
            st_p(1); mark("p.1")
            st_ffn(0, 0); mark("ffn0.0")
            st_ln2(1); mark("ln2.1")
            st_ffn(0, 1); mark("ffn1.0")
            st_f3_out(0); mark("f3.0")
            st_ffn(1, 0); mark("ffn0.1")
            st_ffn(1, 1); mark("ffn1.1")
            st_f3_out(1); mark("f3.1")

    nc.compile()
    names = dict(x=xg.name, out=out.name, G=G_d.name, Wv=Wv_d.name, Wp=Wp_d.name,
                 F1=F1_d.name, F3=F3_d.name, dgp=dgp_d.name, dgs=dgs_d.name,
                 cols=cols_d.name, colff=colff_d.name)
    return nc, names


def _upmat():
    def idx(n, s):
        src = np.maximum((np.arange(n * s) + 0.5) / s - 0.5, 0.0)
        i0 = np.minimum(np.floor(src).astype(np.int64), n - 1)
        i1 = np.minimum(i0 + 1, n - 1)
        return i0, i1, src - i0
    R = np.zeros((RES, SR + 1), np.float64)
    i0, i1, t = idx(SR + 1, SR)
    for y in range(RES):
        R[y, i0[y]] += 1 - t[y]
        R[y, i1[y]] += t[y]
    return np.einsum('yi,xj->ijyx', R, R).reshape(64, N).astype(np.float32)


def _cols(v):
    return np.ascontiguousarray(np.asarray(v, np.float32).reshape(-1, 128).T)


def _dr_w(w):
    """W [M,K] -> DR lhsT [128, K/256 pairs..., M/128, 128] as [128, 2, M/128, 128]
    per 256-K block; returns [128, nkp, 2, nm, 128]? here K=256 -> [128, 2, nm, 128]."""
    M, K = w.shape
    nm = M // 128
    wT = w.T.reshape(2, 128, nm, 128)          # [kc, k, mc, m]
    return np.ascontiguousarray(wT.transpose(1, 0, 2, 3)).astype(F8)


def kernel(**inputs):
    if "prog" not in _CACHE:
        _CACHE["prog"] = _build()
    nc, nm = _CACHE["prog"]
    ii = {k: np.asarray(v) for k, v in inputs.items()}

    inv1 = ii["bn1_g"] / np.sqrt(ii["bn1_v"] + BN_EPS)
    inv2 = ii["bn2_g"] / np.sqrt(ii["bn2_v"] + BN_EPS)
    invv = ii["bnv_g"] / np.sqrt(ii["bnv_v"] + BN_EPS)
    invf1 = ii["bf1_g"] / np.sqrt(ii["bf1_v"] + BN_EPS)
    invf2 = ii["bf2_g"] / np.sqrt(ii["bf2_v"] + BN_EPS)
    invf3 = ii["bf3_g"] / np.sqrt(ii["bf3_v"] + BN_EPS)

    g1, b1 = ii["n1_g"], ii["n1_b"]
    g2, b2 = ii["n2_g"], ii["n2_b"]

    Wv_eff = ii["Wv"] * g1[None, :]
    B_v = ii["Wv"] @ b1
    F1_eff = ii["f1_w"] * g2[None, :]
    betaf1 = invf1 * (ii["f1_w"] @ b2 + ii["f1_b"] - ii["bf1_m"]) + ii["bf1_b"]
    B_p = ii["Wp"] @ (ii["bnv_b"] - ii["bnv_m"] * invv) + ii["bp"]
    betaf3 = invf3 * (ii["f3_b"] - ii["bf3_m"]) + ii["bf3_b"]

    # conv diagonals (raw weights * SW; bn inv applied at drain scale)
    dgp = np.zeros((128, 14, 3, 2, 128), F8)
    dgs = np.zeros((128, 14, 3, 128), F8)
    convw = [ii["c1_w"][:, 0], ii["c2_w"][:, 0], ii["vu_w"][:, 0], ii["f2_w"][:, 0]]
    ci = 0
    ar = np.arange(128)
    for w in convw:
        nch = w.shape[0] // 128
        wr = w.reshape(nch, 128, 3, 3)
        for c in range(nch):
            for x in range(3):
                dgp[ar, ci, x, 0, ar] = (wr[c, :, 0, x] * SW).astype(F8)
                dgp[ar, ci, x, 1, ar] = (wr[c, :, 1, x] * SW).astype(F8)
                dgs[ar, ci, x, ar] = (wr[c, :, 2, x] * SW).astype(F8)
            ci += 1

    beta1 = ii["bn1_b"] - ii["bn1_m"] * inv1
    beta2 = ii["bn2_b"] - ii["bn2_m"] * inv2

    cols = np.zeros((128, 24), np.float32)
    cols[:, 0:2] = _cols(B_v)
    cols[:, 2:4] = _cols(inv1 / SW)
    cols[:, 4:6] = _cols(beta1)
    cols[:, 6:8] = _cols(inv1 / (49.0 * SW))
    cols[:, 8:10] = _cols(inv2 / SW)
    cols[:, 10:12] = _cols(beta2)
    cols[:, 12:14] = _cols(inv2 / (49.0 * SW))
    cols[:, 14:16] = _cols(invv / SW)
    cols[:, 16:18] = _cols(B_p)
    cols[:, 18:20] = _cols(invf3 / SW)
    cols[:, 20:22] = _cols(betaf3)
    # 22/23 unused placeholders (f2 uses colf2 via cols? see colf2 below)

    betaf2 = invf2 * (ii["f2_b"] - ii["bf2_m"]) + ii["bf2_b"]
    colff = np.zeros((128, 4, 8), np.float32)
    colff[:, 0, :] = _cols(invf1 / SW)
    colff[:, 1, :] = _cols(betaf1)
    colff[:, 2, :] = _cols(invf2 / SW)
    colff[:, 3, :] = _cols(betaf2)

    consts = {
        nm["G"]: np.tile(_upmat(), (2, 1)).astype(F8),
        nm["Wv"]: _dr_w(Wv_eff * SW),
        nm["Wp"]: _dr_w(ii["Wp"] * SW),
        nm["F1"]: _dr_w(F1_eff * SW),
        nm["dgp"]: dgp, nm["dgs"]: dgs,
        nm["cols"]: cols, nm["colff"]: colff,
    }
    # F3: [128 k, pair p, kc-in-pair, mc, 128 m]
    f3T = (ii["f3_w"] * SW).T.reshape(4, 2, 128, 2, 128)   # [p, kc, k, mc, m]
    consts[nm["F3"]] = np.ascontiguousarray(f3T.transpose(2, 0, 1, 3, 4)).astype(F8)

    x = np.ascontiguousarray(ii["x"].astype(BF))
    in_maps = [dict(consts, **{nm["x"]: np.ascontiguousarray(x[c * BPC:(c + 1) * BPC])})
               for c in range(NCORES)]
    kw = {}
    if _CACHE.get("trace"):
        import shutil
        shutil.rmtree("/tmp/bass_trace", ignore_errors=True)
        os.makedirs("/tmp/bass_trace", exist_ok=True)
        kw = dict(trace=True, trace_cores=[0], tmpdir="/tmp/bass_trace")
    res = run_bass_kernel_spmd(nc, in_maps, list(range(NCORES)), **kw)
    _CACHE["last_res"] = res
    return np.concatenate([res.results[c][nm["out"]] for c in range(NCORES)],
                          axis=0).astype(np.float32)



# revision 34
# speedup vs baseline: 1.0976x; 1.0527x over previous
import sys, os
sys.path.insert(0, '/opt/trn_rl_repo')
import numpy as np
import ml_dtypes

import concourse.bass as bass
import concourse.mybir as mybir
import concourse.tile as tile
from concourse import bacc
from concourse.bass_utils import run_bass_kernel_spmd

F32 = mybir.dt.float32
BF16 = mybir.dt.bfloat16
FP8 = mybir.dt.float8e4
PM = mybir.MatmulPerfMode.DoubleRow
AF = mybir.ActivationFunctionType
OP = mybir.AluOpType
F8 = ml_dtypes.float8_e4m3fn
BF = ml_dtypes.bfloat16

B, DIM, HEADS, SR, RES, HID = 16, 256, 8, 7, 56, 1024
N = RES * RES              # 3136
NP = 3200                  # padded token count (25*128) for DMA transpose
LN_EPS, BN_EPS = 1e-6, 1e-5
NCORES = 8
BPC = B // NCORES
NT = 25
SW = 16.0                  # fp8 weight rescale

_CACHE = {}


def _build():
    nc = bacc.Bacc(None, target_bir_lowering=False, debug=True)

    xg = nc.dram_tensor([BPC, N, DIM], BF16, kind="ExternalInput")
    out = nc.dram_tensor([BPC, N, DIM], BF16, kind="ExternalOutput")
    G_d = nc.dram_tensor([128, N], FP8, kind="ExternalInput")
    Wv_d = nc.dram_tensor([128, 2, 2, 128], FP8, kind="ExternalInput")
    Wp_d = nc.dram_tensor([128, 2, 2, 128], FP8, kind="ExternalInput")
    F1_d = nc.dram_tensor([128, 2, 8, 128], FP8, kind="ExternalInput")
    F3_d = nc.dram_tensor([128, 4, 2, 2, 128], FP8, kind="ExternalInput")
    dgp_d = nc.dram_tensor([128, 14, 3, 2, 128], FP8, kind="ExternalInput")
    dgs_d = nc.dram_tensor([128, 14, 3, 128], FP8, kind="ExternalInput")
    # per-partition columns: ln g/b not needed (folded); drain scales/biases
    cols_d = nc.dram_tensor([128, 24], F32, kind="ExternalInput")
    colff_d = nc.dram_tensor([128, 4, 8], F32, kind="ExternalInput")

    # cols layout (k index):
    # 0,1: B_v ; 2,3: inv1/S ; 4,5: beta1 ; 6,7: invq1 = inv1/(49 S) ;
    # 8,9: inv2/S ; 10,11: beta2 ; 12,13: invq2 ; 14,15: invv/S ;
    # 16,17: B_p ; 18,19: invf3/S ; 20,21: betaf3 ; 22: unused, 23: unused
    # colf1: [:, 0, :]=invf1/S  [:, 1, :]=betaf1'

    with tile.TileContext(nc) as tc:
        with (
            tc.tile_pool(name="cst", bufs=1) as cst,
            tc.tile_pool(name="big", bufs=1) as big,
            tc.tile_pool(name="sm", bufs=1) as sm,
            tc.tile_pool(name="tmp", bufs=3) as tmp,
            tc.tile_pool(name="pps", bufs=8, space="PSUM") as pps,
        ):
            G = cst.tile([128, N], FP8)
            nc.sync.dma_start(out=G, in_=G_d[:])
            Wv = cst.tile([128, 2, 2, 128], FP8)
            nc.sync.dma_start(out=Wv, in_=Wv_d[:])
            Wp = cst.tile([128, 2, 2, 128], FP8)
            nc.sync.dma_start(out=Wp, in_=Wp_d[:])
            F1 = cst.tile([128, 2, 8, 128], FP8)
            nc.sync.dma_start(out=F1, in_=F1_d[:])
            F3 = cst.tile([128, 4, 2, 2, 128], FP8)
            nc.sync.dma_start(out=F3, in_=F3_d[:])
            dgp = cst.tile([128, 14, 3, 2, 128], FP8)
            nc.sync.dma_start(out=dgp, in_=dgp_d[:])
            dgs = cst.tile([128, 14, 3, 128], FP8)
            nc.sync.dma_start(out=dgs, in_=dgs_d[:])
            cols = cst.tile([128, 24], F32)
            nc.sync.dma_start(out=cols, in_=cols_d[:])
            colff = cst.tile([128, 4, 8], F32)
            nc.sync.dma_start(out=colff, in_=colff_d[:])
            Cmagic = cst.tile([128, 32], mybir.dt.int32)
            nc.vector.memset(Cmagic, 0x5f3759df)

            # persistent padded conv-input buffers (borders stay zero)
            v_pad = cst.tile([128, 2, 58, 64], FP8)
            nc.vector.memset(v_pad.bitcast(mybir.dt.int32), 0)
            s1_pad = cst.tile([128, 2, 58, 64], FP8)
            nc.vector.memset(s1_pad.bitcast(mybir.dt.int32), 0)
            yu_pad = cst.tile([128, 2, 58, 64], FP8)
            nc.vector.memset(yu_pad.bitcast(mybir.dt.int32), 0)
            z1_pad = cst.tile([128, 4, 58, 64], FP8)
            nc.vector.memset(z1_pad.bitcast(mybir.dt.int32), 0)
            v_aug = cst.tile([128, NT, 4, 66], BF16)
            nc.vector.memset(v_aug[:, :, :, 32:33], 1.0)
            nc.vector.memset(v_aug[:, :, :, 65:66], 1.0)

            def col(k):
                return cols[:, k:k + 1]

            def ln_to_ct(x_tok, xn_tok, xn_cti, xn_fp8):
                """batched LN: DVE-only stats (bit-trick rsqrt + 1 Newton step),
                normalize -> bf16 xn_tok; DMA-T -> bf16 xn_cti; scalar Copy cast
                -> plain fp8 [128,2,NP]. Generator: yields between phases."""
                s1 = tmp.tile([128, NT], F32, tag="s1", bufs=2, name="s1")
                nc.vector.tensor_reduce(out=s1, in_=x_tok,
                                        axis=mybir.AxisListType.X, op=OP.add)
                nc.vector.tensor_tensor(out=xn_tok, in0=x_tok, in1=x_tok,
                                        op=OP.mult)
                s2 = tmp.tile([128, NT], F32, tag="s2", bufs=2, name="s2")
                nc.vector.tensor_reduce(out=s2, in_=xn_tok,
                                        axis=mybir.AxisListType.X, op=OP.add)
                yield
                m = tmp.tile([128, NT], F32, tag="lnm", bufs=2, name="m")
                nc.vector.tensor_scalar(out=m, in0=s1, scalar1=1.0 / 256,
                                        scalar2=None, op0=OP.mult)
                w = tmp.tile([128, NT], F32, tag="lnw", bufs=2, name="w")
                nc.vector.tensor_scalar(out=w, in0=s2, scalar1=1.0 / 256,
                                        scalar2=LN_EPS, op0=OP.mult, op1=OP.add)
                mm2 = tmp.tile([128, NT], F32, tag="lnm2", bufs=2, name="mm2")
                nc.vector.tensor_tensor(out=mm2, in0=m, in1=m, op=OP.mult)
                nc.vector.tensor_tensor(out=w, in0=w, in1=mm2, op=OP.subtract)
                yi = tmp.tile([128, NT], mybir.dt.int32, tag="lnyi", bufs=2,
                              name="yi")
                nc.vector.tensor_scalar(out=yi, in0=w.bitcast(mybir.dt.int32),
                                        scalar1=1, scalar2=None,
                                        op0=OP.logical_shift_right)
                nc.vector.tensor_tensor(out=yi, in0=Cmagic[:, 0:NT], in1=yi,
                                        op=OP.subtract)
                y0 = yi.bitcast(F32)
                a = tmp.tile([128, NT], F32, tag="lna", bufs=2, name="a")
                nc.vector.tensor_tensor(out=a, in0=w, in1=y0, op=OP.mult)
                nc.vector.tensor_tensor(out=a, in0=a, in1=y0, op=OP.mult)
                nc.vector.tensor_scalar(out=a, in0=a, scalar1=-0.5, scalar2=1.5,
                                        op0=OP.mult, op1=OP.add)
                rs = tmp.tile([128, NT], F32, tag="lnrs", bufs=2, name="rs")
                nc.vector.tensor_tensor(out=rs, in0=y0, in1=a, op=OP.mult)
                nm = tmp.tile([128, NT], F32, tag="lnnm", bufs=2, name="nm")
                nc.vector.tensor_tensor(out=nm, in0=m, in1=rs, op=OP.mult)
                yield
                srcv = xn_cti.rearrange("p (t k j) -> p t k j", k=2, j=128)
                for q0, q1 in ((0, 7), (7, 13), (13, 19), (19, 25)):
                    for t in range(q0, q1):
                        rows = 128 if t < NT - 1 else 64
                        nc.vector.tensor_scalar(
                            out=xn_tok[:rows, t, :], in0=x_tok[:rows, t, :],
                            scalar1=rs[:rows, t:t + 1], scalar2=nm[:rows, t:t + 1],
                            op0=OP.mult, op1=OP.subtract)
                    nc.sync.dma_start_transpose(
                        out=xn_cti[:, q0 * 256:q1 * 256].rearrange(
                            "p (m j) -> p m j", j=128),
                        in_=xn_tok[:, q0:q1, :].rearrange("p t c -> p (t c)"))
                    tend = min(q1, 24)
                    nc.scalar.copy(
                        out=xn_fp8[:, :, q0 * 128:tend * 128].rearrange(
                            "p k (t j) -> p k t j", j=128),
                        in_=srcv[:, q0:tend].rearrange("p t k j -> p k t j"))
                    yield
                nc.scalar.copy(out=xn_fp8[:, :, 3072:3136], in_=srcv[:, 24, :, 0:64])

            def conv(pad_ch, ci, nrows, drain):
                """fp8-DR conv on padded [58,64] image chunk. pairs ((0,x),(1,x)) + singles
                (2,x). nrows: stripe rows (7 for pooled convs, 8 otherwise)."""
                nstripe = RES // nrows
                for s in range(nstripe):
                    r0 = s * nrows
                    cp = pps.tile([128, nrows, RES], F32, tag="cp", bufs=3,
                                  name="cp")
                    for x in range(3):
                        base = pad_ch[:, r0:r0 + nrows, x:x + RES]
                        rhs = base.copy()
                        rhs.ap.insert(1, (64, 2))
                        nc.tensor.matmul(cp, dgp[:, ci, x, :, :], rhs,
                                         start=(x == 0), stop=False, perf_mode=PM)
                    for x in range(3):
                        base = pad_ch[:, r0 + 2:r0 + 2 + nrows, x:x + RES]
                        nc.tensor.matmul(cp, dgs[:, ci, x, :], base,
                                         start=False, stop=(x == 2))
                    drain(s, r0, cp)
                    yield

            def pool_from(cp, pool_out, ch, s, sc_k):
                t2 = tmp.tile([128, 8], F32, tag="t2", bufs=4, name="t2")
                nc.vector.tensor_reduce(
                    out=t2, in_=cp.rearrange("p h (wb k) -> p wb h k", k=7),
                    axis=mybir.AxisListType.XY, op=OP.add)
                nc.vector.tensor_scalar(out=pool_out[:, ch, s, :], in0=t2,
                                        scalar1=col(sc_k + ch), scalar2=col(sc_k - 2 + ch),
                                        op0=OP.mult, op1=OP.add)

            S = [dict() for _ in range(BPC)]

            def st_load(b):
                d = S[b]
                x_tok = big.tile([128, NT, 256], BF16, tag="x_tok", bufs=2,
                                 name=f"x_tok{b}")
                nc.sync.dma_start(
                    out=x_tok[:, 0:24, :],
                    in_=xg[b, 0:3072, :].rearrange("(t p) c -> p t c", p=128))
                nc.sync.dma_start(out=x_tok[:64, 24, :], in_=xg[b, 3072:3136, :])
                d.update(x_tok=x_tok)

            def g_ln1(b):
                d = S[b]
                xn_tok = big.tile([128, NT, 256], BF16, tag="xn_tok", bufs=2,
                                  name=f"xn_tok{b}")
                xn_cti = big.tile([128, NT * 256], BF16, tag="shA", name=f"xn_cti{b}")
                xn_fp8 = big.tile([128, 2, N], FP8, tag="shC", name=f"xn_fp8{b}")
                d.update(xn_fp8=xn_fp8)
                yield from ln_to_ct(d["x_tok"], xn_tok, xn_cti, xn_fp8)

            def g_wv(b):
                d = S[b]
                v_dense = big.tile([128, 2, NP], BF16, tag="shA", name=f"v_dense{b}")
                d.update(v_dense=v_dense)
                for mc in range(2):
                    for s in range(7):
                        pv = pps.tile([128, 448], F32, tag="pgA", bufs=2, name="pv")
                        nc.tensor.matmul(pv, Wv[:, :, mc, :],
                                         d["xn_fp8"][:, :, s * 448:(s + 1) * 448],
                                         start=True, stop=True, perf_mode=PM)
                        nc.vector.tensor_scalar(
                            out=v_dense[:, mc, s * 448:(s + 1) * 448], in0=pv,
                            scalar1=1.0 / SW, scalar2=col(0 + mc),
                            op0=OP.mult, op1=OP.add)
                        nc.scalar.activation(
                            out=v_pad[:, mc, 1 + 8 * s:9 + 8 * s, 1:57], in_=pv,
                            func=AF.Identity, scale=1.0 / SW, bias=col(0 + mc))
                        yield

            def g_c1c2(b):
                d = S[b]
                qv = sm.tile([128, 2, 8, 8], F32, tag="qv", name=f"qv{b}")
                kv = sm.tile([128, 2, 8, 8], F32, tag="kv", name=f"kv{b}")
                skip2 = big.tile([128, 2, NP], BF16, tag="xn_tok", bufs=2,
                                 name=f"skip2{b}")
                d.update(qv=qv, kv=kv, skip2=skip2)
                for ch in range(2):
                    def drain1(s, r0, cp, ch=ch):
                        pool_from(cp, qv, ch, s, 6)
                        nc.scalar.activation(
                            out=s1_pad[:, ch, 1 + r0:1 + r0 + 7, 1:57],
                            in_=cp, func=AF.Gelu, scale=col(2 + ch), bias=col(4 + ch))
                    yield from conv(v_pad[:, ch], 0 + ch, 7, drain1)
                for ch in range(2):
                    def drain2(s, r0, cp, ch=ch):
                        pool_from(cp, kv, ch, s, 12)
                        nc.scalar.activation(
                            out=skip2[:, ch, r0 * RES:(r0 + 7) * RES],
                            in_=cp, func=AF.Gelu, scale=col(8 + ch), bias=col(10 + ch))
                    yield from conv(s1_pad[:, ch], 2 + ch, 7, drain2)

            def g_at(b):
                d = S[b]
                v_tok = big.tile([128, 2, NT, 128], BF16, tag="v_tok", name=f"v_tok{b}")
                for kc in range(2):
                    nc.sync.dma_start_transpose(out=v_tok[:, kc], in_=d["v_dense"][:, kc])
                yield
                for kc in range(2):
                    nc.vector.tensor_copy(
                        out=v_aug[:, :, 2 * kc:2 * kc + 2, :].rearrange(
                            "p t i (s d) -> p t i s d", s=2)[:, :, :, :, 0:32],
                        in_=v_tok[:, kc].rearrange("p t (i s d) -> p t i s d",
                                                   i=2, s=2))
                yield
                qb = sm.tile([128, 2, 64], BF16, tag="qb", name="qb")
                nc.vector.tensor_copy(out=qb, in_=d["qv"].rearrange("p c h w -> p c (h w)"))
                kb = sm.tile([128, 2, 64], BF16, tag="kb", name="kb")
                nc.vector.tensor_copy(out=kb, in_=d["kv"].rearrange("p c h w -> p c (h w)"))
                qb0 = sm.tile([32, 8, 64], BF16, tag="qb0", name="qb0")
                kb0 = sm.tile([32, 8, 64], BF16, tag="kb0", name="kb0")
                for h in range(8):
                    ch, off = h // 4, (h % 4) * 32
                    nc.vector.tensor_copy(out=qb0[:, h, :], in_=qb[off:off + 32, ch, :])
                    nc.vector.tensor_copy(out=kb0[:, h, :], in_=kb[off:off + 32, ch, :])
                pqk = pps.tile([64, 8, 64], F32, tag="pa", bufs=1, name="pqk")
                for h in range(8):
                    nc.tensor.matmul(pqk[:, h, :], kb0[:, h, :], qb0[:, h, :],
                                     start=(h == 0), stop=(h == 7))
                a2t = sm.tile([64, 8, 64], FP8, tag="a2t", name="a2t")
                nc.scalar.copy(out=a2t, in_=pqk)
                yield
                pys = pps.tile([128, 4, 66], F32, tag="pa", bufs=1, name="pys")
                for t in range(NT):
                    K = 128 if t < NT - 1 else 64
                    pe = pps.tile([128, 512], F32, tag="pgB", bufs=2, name="pe")
                    nc.tensor.matmul(pe[:K, :], G[0:64, t * 128:t * 128 + K],
                                     a2t.rearrange("p h q -> p (h q)"),
                                     start=True, stop=True)
                    eT = tmp.tile([128, 512], BF16, tag="eT", bufs=2, name="eT")
                    nc.scalar.activation(out=eT[:K], in_=pe[:K], func=AF.Exp,
                                         scale=float(DIM) ** -0.5)
                    for i in range(4):
                        nc.tensor.matmul(pys[:, i, :], eT[:K, i * 128:(i + 1) * 128],
                                         v_aug[:K, t, i, :],
                                         start=(t == 0 and i == 0),
                                         stop=(t == NT - 1 and i == 3))
                    yield
                rz = sm.tile([128, 4], F32, tag="rz", name="rz")
                nc.vector.reciprocal(out=rz[0:64], in_=pys[0:64, :, 32])
                nc.vector.reciprocal(out=rz[64:128], in_=pys[64:128, :, 65])
                y_rT = sm.tile([64, 256], FP8, tag="y_rT", name="y_rT")
                yv = y_rT.rearrange("p (i x d) -> p i x d", i=4, x=2)
                for i in range(4):
                    nc.vector.tensor_scalar(
                        out=yv[:, i, 0, :], in0=pys[0:64, i, 0:32],
                        scalar1=rz[0:64, i:i + 1], scalar2=None, op0=OP.mult)
                y_odd = sm.tile([128, 4, 32], FP8, tag="y_odd", name="y_odd")
                for i in range(4):
                    nc.vector.tensor_scalar(
                        out=y_odd[64:128, i, :], in0=pys[64:128, i, 33:65],
                        scalar1=rz[64:128, i:i + 1], scalar2=None, op0=OP.mult)
                nc.sync.dma_start(out=yv[:, :, 1, :], in_=y_odd[64:128])
                yield
                for ch in range(2):
                    for s in range(7):
                        pu = pps.tile([128, 448], F32, tag="pgB", bufs=2, name="pu")
                        nc.tensor.matmul(pu, y_rT[:, ch * 128:(ch + 1) * 128],
                                         G[0:64, s * 448:(s + 1) * 448],
                                         start=True, stop=True)
                        nc.vector.tensor_copy(
                            out=yu_pad[:, ch, 1 + 8 * s:9 + 8 * s, 1:57], in_=pu)
                        yield

            def g_vu(b):
                d = S[b]
                ysum = big.tile([128, 2, N], FP8, tag="ysum", name=f"ysum{b}")
                d.update(ysum=ysum)
                for ch in range(2):
                    def drainv(s, r0, cp, ch=ch):
                        nc.vector.scalar_tensor_tensor(
                            out=ysum[:, ch, r0 * RES:(r0 + 8) * RES],
                            in0=cp.rearrange("p a b -> p (a b)"),
                            scalar=col(14 + ch),
                            in1=d["skip2"][:, ch, r0 * RES:(r0 + 8) * RES],
                            op0=OP.mult, op1=OP.add)
                    yield from conv(yu_pad[:, ch], 4 + ch, 8, drainv)

            def g_p(b):
                d = S[b]
                p_ct = big.tile([128, 2, NP], BF16, tag="shB", name=f"p_ct{b}")
                for mc in range(2):
                    for s in range(7):
                        pp = pps.tile([128, 448], F32, tag="pgB", bufs=2, name="pp")
                        nc.tensor.matmul(pp, Wp[:, :, mc, :],
                                         d["ysum"][:, :, s * 448:(s + 1) * 448],
                                         start=True, stop=True, perf_mode=PM)
                        nc.vector.tensor_scalar(
                            out=p_ct[:, mc, s * 448:(s + 1) * 448], in0=pp,
                            scalar1=1.0 / SW, scalar2=col(16 + mc),
                            op0=OP.mult, op1=OP.add)
                    yield
                p_tok = big.tile([128, 2, NT, 128], BF16, tag="v_tok", name=f"p_tok{b}")
                for kc in range(2):
                    nc.sync.dma_start_transpose(out=p_tok[:, kc], in_=p_ct[:, kc])
                    nc.vector.tensor_tensor(
                        out=d["x_tok"][:, :, kc * 128:(kc + 1) * 128],
                        in0=d["x_tok"][:, :, kc * 128:(kc + 1) * 128],
                        in1=p_tok[:, kc], op=OP.add)
                    yield

            def g_ln2(b):
                d = S[b]
                xn2_tok = big.tile([128, NT, 256], BF16, tag="xn_tok", bufs=2,
                                   name=f"xn2_tok{b}")
                xn2_cti = big.tile([128, NT * 256], BF16, tag="shA", name=f"xn2_cti{b}")
                xn2_fp8 = big.tile([128, 2, N], FP8, tag="shC", name=f"xn2_fp8{b}")
                d.update(xn2_fp8=xn2_fp8)
                yield from ln_to_ct(d["x_tok"], xn2_tok, xn2_cti, xn2_fp8)

            def g_f1f2(b, half):
                d = S[b]
                if half == 0:
                    d["z2"] = big.tile([128, 8, N], FP8, tag="shZ", name=f"z2{b}")
                z2 = d["z2"]
                for m4 in range(4):
                    mc = half * 4 + m4
                    for s in range(7):
                        pf = pps.tile([128, 448], F32, tag="pgA", bufs=2, name="pf")
                        nc.tensor.matmul(pf, F1[:, :, mc, :],
                                         d["xn2_fp8"][:, :, s * 448:(s + 1) * 448],
                                         start=True, stop=True, perf_mode=PM)
                        nc.scalar.activation(
                            out=z1_pad[:, m4, 1 + 8 * s:9 + 8 * s, 1:57],
                            in_=pf, func=AF.Gelu, scale=colff[:, 0, mc:mc + 1],
                            bias=colff[:, 1, mc:mc + 1])
                    yield
                for m4 in range(4):
                    hc = half * 4 + m4
                    def drainf(s, r0, cp, hc=hc):
                        nc.scalar.activation(
                            out=z2[:, hc, r0 * RES:(r0 + 8) * RES],
                            in_=cp.rearrange("p a b -> p (a b)"), func=AF.Gelu,
                            scale=colff[:, 2, hc:hc + 1], bias=colff[:, 3, hc:hc + 1])
                    yield from conv(z1_pad[:, m4], 6 + hc, 8, drainf)

            def g_f3(b):
                d = S[b]
                z3_ct = big.tile([128, 2, NP], BF16, tag="shB", name=f"z3_ct{b}")
                for mc in range(2):
                    for s in range(7):
                        pf3 = pps.tile([128, 448], F32, tag="pgB", bufs=2, name="pf3")
                        for p in range(4):
                            nc.tensor.matmul(pf3, F3[:, p, :, mc, :],
                                             d["z2"][:, 2 * p:2 * p + 2,
                                                     s * 448:(s + 1) * 448],
                                             start=(p == 0), stop=(p == 3), perf_mode=PM)
                        nc.vector.tensor_scalar(
                            out=z3_ct[:, mc, s * 448:(s + 1) * 448], in0=pf3,
                            scalar1=col(18 + mc), scalar2=col(20 + mc),
                            op0=OP.mult, op1=OP.add)
                        yield
                z3_tok = big.tile([128, 2, NT, 128], BF16, tag="v_tok", name=f"z3_tok{b}")
                for kc in range(2):
                    nc.sync.dma_start_transpose(out=z3_tok[:, kc], in_=z3_ct[:, kc])
                    nc.vector.tensor_tensor(
                        out=d["x_tok"][:, :, kc * 128:(kc + 1) * 128],
                        in0=d["x_tok"][:, :, kc * 128:(kc + 1) * 128],
                        in1=z3_tok[:, kc], op=OP.add)
                    yield
                nc.sync.dma_start(
                    out=out[b, 0:3072, :].rearrange("(t p) c -> p t c", p=128),
                    in_=d["x_tok"][:, 0:24, :])
                nc.sync.dma_start(out=out[b, 3072:3136, :], in_=d["x_tok"][:64, 24, :])

            # interleaved generator-driven emission across the 2 batch elements:
            # keeps the PE fed during scalar/DVE-heavy phases and groups scalar
            # activations by table set (Gelu windows vs Exp windows; Copy is free).
            from itertools import chain as CH

            def RR(*gens):
                gens = list(gens)
                while gens:
                    nxt = []
                    for gg in gens:
                        try:
                            next(gg)
                            nxt.append(gg)
                        except StopIteration:
                            pass
                    gens = nxt

            def drain(gg):
                for _ in gg:
                    pass

            st_load(0)
            st_load(1)
            drain(g_ln1(0))
            RR(g_wv(0), g_ln1(1))
            drain(g_c1c2(0))
            RR(g_at(0), g_wv(1))
            drain(g_c1c2(1))
            RR(g_at(1), g_vu(0))
            RR(CH(g_p(0), g_ln2(0)), g_vu(1))
            RR(CH(g_f1f2(0, 0), g_f1f2(0, 1)), CH(g_p(1), g_ln2(1)))
            RR(CH(g_f1f2(1, 0), g_f1f2(1, 1)), g_f3(0))
            drain(g_f3(1))

    nc.compile()
    names = dict(x=xg.name, out=out.name, G=G_d.name, Wv=Wv_d.name, Wp=Wp_d.name,
                 F1=F1_d.name, F3=F3_d.name, dgp=dgp_d.name, dgs=dgs_d.name,
                 cols=cols_d.name, colff=colff_d.name)
    return nc, names


def _upmat():
    def idx(n, s):
        src = np.maximum((np.arange(n * s) + 0.5) / s - 0.5, 0.0)
        i0 = np.minimum(np.floor(src).astype(np.int64), n - 1)
        i1 = np.minimum(i0 + 1, n - 1)
        return i0, i1, src - i0
    R = np.zeros((RES, SR + 1), np.float64)
    i0, i1, t = idx(SR + 1, SR)
    for y in range(RES):
        R[y, i0[y]] += 1 - t[y]
        R[y, i1[y]] += t[y]
    return np.einsum('yi,xj->ijyx', R, R).reshape(64, N).astype(np.float32)


def _cols(v):
    return np.ascontiguousarray(np.asarray(v, np.float32).reshape(-1, 128).T)


def _dr_w(w):
    """W [M,K] -> DR lhsT [128, K/256 pairs..., M/128, 128] as [128, 2, M/128, 128]
    per 256-K block; returns [128, nkp, 2, nm, 128]? here K=256 -> [128, 2, nm, 128]."""
    M, K = w.shape
    nm = M // 128
    wT = w.T.reshape(2, 128, nm, 128)          # [kc, k, mc, m]
    return np.ascontiguousarray(wT.transpose(1, 0, 2, 3)).astype(F8)


def kernel(**inputs):
    if "prog" not in _CACHE:
        _CACHE["prog"] = _build()
    nc, nm = _CACHE["prog"]
    ii = {k: np.asarray(v) for k, v in inputs.items()}

    inv1 = ii["bn1_g"] / np.sqrt(ii["bn1_v"] + BN_EPS)
    inv2 = ii["bn2_g"] / np.sqrt(ii["bn2_v"] + BN_EPS)
    invv = ii["bnv_g"] / np.sqrt(ii["bnv_v"] + BN_EPS)
    invf1 = ii["bf1_g"] / np.sqrt(ii["bf1_v"] + BN_EPS)
    invf2 = ii["bf2_g"] / np.sqrt(ii["bf2_v"] + BN_EPS)
    invf3 = ii["bf3_g"] / np.sqrt(ii["bf3_v"] + BN_EPS)

    g1, b1 = ii["n1_g"], ii["n1_b"]
    g2, b2 = ii["n2_g"], ii["n2_b"]

    Wv_eff = ii["Wv"] * g1[None, :]
    B_v = ii["Wv"] @ b1
    F1_eff = ii["f1_w"] * g2[None, :]
    betaf1 = invf1 * (ii["f1_w"] @ b2 + ii["f1_b"] - ii["bf1_m"]) + ii["bf1_b"]
    B_p = ii["Wp"] @ (ii["bnv_b"] - ii["bnv_m"] * invv) + ii["bp"]
    betaf3 = invf3 * (ii["f3_b"] - ii["bf3_m"]) + ii["bf3_b"]

    # conv diagonals (raw weights * SW; bn inv applied at drain scale)
    dgp = np.zeros((128, 14, 3, 2, 128), F8)
    dgs = np.zeros((128, 14, 3, 128), F8)
    convw = [ii["c1_w"][:, 0], ii["c2_w"][:, 0], ii["vu_w"][:, 0], ii["f2_w"][:, 0]]
    ci = 0
    ar = np.arange(128)
    for w in convw:
        nch = w.shape[0] // 128
        wr = w.reshape(nch, 128, 3, 3)
        for c in range(nch):
            for x in range(3):
                dgp[ar, ci, x, 0, ar] = (wr[c, :, 0, x] * SW).astype(F8)
                dgp[ar, ci, x, 1, ar] = (wr[c, :, 1, x] * SW).astype(F8)
                dgs[ar, ci, x, ar] = (wr[c, :, 2, x] * SW).astype(F8)
            ci += 1

    beta1 = ii["bn1_b"] - ii["bn1_m"] * inv1
    beta2 = ii["bn2_b"] - ii["bn2_m"] * inv2

    cols = np.zeros((128, 24), np.float32)
    cols[:, 0:2] = _cols(B_v)
    cols[:, 2:4] = _cols(inv1 / SW)
    cols[:, 4:6] = _cols(beta1)
    cols[:, 6:8] = _cols(inv1 / (49.0 * SW))
    cols[:, 8:10] = _cols(inv2 / SW)
    cols[:, 10:12] = _cols(beta2)
    cols[:, 12:14] = _cols(inv2 / (49.0 * SW))
    cols[:, 14:16] = _cols(invv / SW)
    cols[:, 16:18] = _cols(B_p)
    cols[:, 18:20] = _cols(invf3 / SW)
    cols[:, 20:22] = _cols(betaf3)
    # 22/23 unused placeholders (f2 uses colf2 via cols? see colf2 below)

    betaf2 = invf2 * (ii["f2_b"] - ii["bf2_m"]) + ii["bf2_b"]
    colff = np.zeros((128, 4, 8), np.float32)
    colff[:, 0, :] = _cols(invf1 / SW)
    colff[:, 1, :] = _cols(betaf1)
    colff[:, 2, :] = _cols(invf2 / SW)
    colff[:, 3, :] = _cols(betaf2)

    consts = {
        nm["G"]: np.tile(_upmat(), (2, 1)).astype(F8),
        nm["Wv"]: _dr_w(Wv_eff * SW),
        nm["Wp"]: _dr_w(ii["Wp"] * SW),
        nm["F1"]: _dr_w(F1_eff * SW),
        nm["dgp"]: dgp, nm["dgs"]: dgs,
        nm["cols"]: cols, nm["colff"]: colff,
    }
    # F3: [128 k, pair p, kc-in-pair, mc, 128 m]
    f3T = (ii["f3_w"] * SW).T.reshape(4, 2, 128, 2, 128)   # [p, kc, k, mc, m]
    consts[nm["F3"]] = np.ascontiguousarray(f3T.transpose(2, 0, 1, 3, 4)).astype(F8)

    x = np.ascontiguousarray(ii["x"].astype(BF))
    in_maps = [dict(consts, **{nm["x"]: np.ascontiguousarray(x[c * BPC:(c + 1) * BPC])})
               for c in range(NCORES)]
    kw = {}
    if _CACHE.get("trace"):
        import shutil
        shutil.rmtree("/tmp/bass_trace", ignore_errors=True)
        os.makedirs("/tmp/bass_trace", exist_ok=True)
        kw = dict(trace=True, trace_cores=[0], tmpdir="/tmp/bass_trace")
    res = run_bass_kernel_spmd(nc, in_maps, list(range(NCORES)), **kw)
    _CACHE["last_res"] = res
    return np.concatenate([res.results[c][nm["out"]] for c in range(NCORES)],
                          axis=0).astype(np.float32)

